# revision 45
# baseline (speedup 1.0000x reference)
"""Trainium2 Bass kernel: dark-channel + 15x15 erosion (min-pool, stride 1,
+inf padding), data-parallel over 8 NeuronCores.

Input  I: [32, 3, 512, 512] f32, k: scalar (15)
Output:   [32, 1, 512, 512] f32  (min over channels, then kxk spatial min)

Per-core plan (4 images each):
  1. DMA the image (3 channels, one transfer) into SBUF, rows on partitions.
  2. Channel min on GpSimd (2 tensor_tensor min ops) -> padded f16 buffer.
  3. Horizontal 15-min-filter on DVE: dyadic shifted mins (1,2,4,7).
  4. PE transpose (identity matmul), 4 blocks per PSUM bank, one ScalarE
     evac per bank -> column layout.
  5. Vertical 15-min-filter on DVE (same dyadic trick along free dim).
  6. PE transpose back + ScalarE evac (f16 -> f32 cast) -> row layout.
  7. DMA result to HBM.

fp16 intermediates: values are mins of uniform[0,1) data; min is selection,
not arithmetic, so fp16 keeps rel err ~1e-4. Pad value 30000.0 acts as +inf
for this data range.

Sync-wait budget: the walrus backend encodes at most ONE sync-wait on most
compute instructions (TensorTensor/Activation/Ldweights/Memset) and fails
codegen with "Too many sync wait commands" otherwise.  Tile emits extra
waits on the first accessor whenever a pool SLOT is reused (old readers +
old writer must be observed).  This kernel therefore gives every tile a
fresh slot for the whole program (SBUF is large enough for all 4 images'
working set), so only true producer->consumer edges remain - one wait
each.  The PE warm-up transpose absorbs the identity-matrix dependency so
later Ldweights only wait on their own input.  PSUM banks do rotate
(8 banks, 8 groups/image); the resulting second wait lands on Matmult
instructions, which accept two waits.
"""

import sys

if "/opt/trn_rl_repo" not in sys.path:
    sys.path.insert(0, "/opt/trn_rl_repo")

import numpy as np

N_CORES = 8
IMGS = 4          # images per core
C = 3
H = W = 512
K = 15
PAD = K // 2      # 7
L = 8             # left pad in filter buffers (>= PAD+1, power of 2)
PITCH = L + 512 + 8   # 528, padded row/col length
NJ = H // 128     # row tiles
NB = W // 128     # col blocks
PADV = 30000.0    # effective +inf for data in [0,1)

_cache = {}


def _build_nc(use_f16=True):
    import concourse.bass as bass
    import concourse.mybir as mybir
    import concourse.tile as tile
    import concourse.masks as masks

    F32 = mybir.dt.float32
    FI = mybir.dt.float16 if use_f16 else F32
    MIN = mybir.AluOpType.min

    nc = bass.Bass("TRN2", target_bir_lowering=False, debug=False)
    inp = nc.dram_tensor("inp", [IMGS, C, H, W], F32, kind="ExternalInput")
    out = nc.dram_tensor("out", [IMGS, 1, H, W], F32, kind="ExternalOutput")

    def dyadic(nc, pool, poolb, respool, src, n, i):
        """15-wide min filter along last dim of src [128, n, PITCH];
        logical x at [L : L+512].  Returns [128, n, 512].
        fa/fb scratch is reused only by DVE itself (same-engine waits);
        res gets a fresh slot every call (PE reads it)."""
        f2 = pool.tile([128, n, PITCH], FI, tag="fa", name="f2")
        nc.vector.tensor_tensor(
            f2[:, :, 0:526], src[:, :, 0:526], src[:, :, 1:527], op=MIN
        )
        f4 = poolb.tile([128, n, PITCH], FI, tag="fb", name="f4")
        nc.vector.tensor_tensor(
            f4[:, :, 0:524], f2[:, :, 0:524], f2[:, :, 2:526], op=MIN
        )
        f8 = pool.tile([128, n, PITCH], FI, tag="fa", name="f8")
        nc.vector.tensor_tensor(
            f8[:, :, 0:520], f4[:, :, 0:520], f4[:, :, 4:524], op=MIN
        )
        res = respool.tile([128, n, 512], FI, tag=f"res{i}", name="res")
        nc.vector.tensor_tensor(
            res[:], f8[:, :, 1:513], f8[:, :, 8:520], op=MIN
        )
        return res

    with tile.TileContext(nc) as tc:
        with (
            tc.tile_pool(name="const", bufs=1) as cpool,
            tc.tile_pool(name="io", bufs=1) as io_pool,
            tc.tile_pool(name="hv", bufs=1) as hv,       # fresh per tag
            tc.tile_pool(name="dy", bufs=2) as dy,       # fa: DVE-only scratch
            tc.tile_pool(name="dyb", bufs=1) as dyb,     # fb: no cross-engine input
            tc.tile_pool(name="resp", bufs=1) as resp,
            tc.tile_pool(name="opool", bufs=1) as opool,
            tc.tile_pool(name="psum", bufs=1, space="PSUM") as psum,
        ):
            ident = cpool.tile([128, 128], FI)
            masks.make_identity(nc, ident[:])

            # PE warm-up: one throwaway transpose absorbs the dependency
            # on the identity matrix, so every later Ldweights carries
            # only its own input's semaphore.  It also offsets the PSUM
            # bank rotation so every in-image group reuse is cross-image.
            wpt = psum.tile([128, 2 * NJ, 128], FI, tag="pt0", name="wpt")
            nc.tensor.transpose(wpt[:, 0, :], ident[:], ident[:])

            # --- loads: one DMA per image PAIR ((i c j w) merges into a
            # single AP dim).  Fewer DMAs = fewer DMA-HW semaphore procs,
            # which keeps the kernel-tail drain under the sync-wait cap.
            in_pairs = []
            for q in range(IMGS // 2):
                t = io_pool.tile([128, 2, C, NJ, W], F32, tag=f"in{q}",
                                 name=f"in{q}")
                nc.sync.dma_start(
                    t[:],
                    inp[2 * q : 2 * q + 2].rearrange(
                        "i c (j p) w -> p i c j w", p=128
                    ),
                )
                in_pairs.append(t)
            o_pairs = [
                opool.tile([128, 2, NJ, W], F32, tag=f"op{q}", name=f"op{q}")
                for q in range(IMGS // 2)
            ]

            for i in range(IMGS):
                in_t = in_pairs[i // 2][:, i % 2]

                # --- channel min (GpSimd) -> xpad f16 [128, NJ, PITCH]
                # min(c0,c1) lands in the xpad interior, then the second
                # min folds c2 in place (same-index streaming, no hazard).
                xpad = hv.tile([128, NJ, PITCH], FI, tag=f"xp{i}",
                               name="xpad")
                nc.gpsimd.memset(xpad[:, :, 0:L], PADV)
                nc.gpsimd.memset(xpad[:, :, L + W : PITCH], PADV)
                nc.gpsimd.tensor_tensor(
                    xpad[:, :, L : L + W], in_t[:, 0, :, :],
                    in_t[:, 1, :, :], op=MIN
                )
                nc.gpsimd.tensor_tensor(
                    xpad[:, :, L : L + W], xpad[:, :, L : L + W],
                    in_t[:, 2, :, :], op=MIN
                )

                # --- horizontal filter (DVE)
                r = dyadic(nc, dy, dyb, resp, xpad, NJ, 2 * i)

                # --- transpose to column layout; 4 blocks (all j for one
                # b) fill one PSUM bank, ONE ACT evac per bank.
                vb = hv.tile([128, NB, PITCH], FI, tag=f"vb{i}", name="vb")
                nc.gpsimd.memset(vb[:, :, 0:L], PADV)
                nc.gpsimd.memset(vb[:, :, L + H : PITCH], PADV)
                for b in range(NB):
                    pt = psum.tile([128, 2 * NJ, 128], FI, tag=f"pt{b}",
                                   name="pt")
                    for j in range(NJ):
                        nc.tensor.transpose(
                            pt[:, j, :], r[:, j, 128 * b : 128 * (b + 1)],
                            ident[:],
                        )
                    nc.vector.tensor_copy(
                        vb[:, b, L : L + H],
                        pt[:, 0:NJ, :].rearrange("p n w -> p (n w)"),
                    )

                # --- vertical filter (DVE)
                u = dyadic(nc, dy, dyb, resp, vb, NB, 2 * i + 1)

                # --- transpose back, f32 out
                o = o_pairs[i // 2][:, i % 2]
                for j in range(NJ):
                    pt = psum.tile([128, 2 * NB, 128], FI, tag=f"pt{4 + j}",
                                   name="pt")
                    for b in range(NB):
                        nc.tensor.transpose(
                            pt[:, b, :], u[:, b, 128 * j : 128 * (j + 1)],
                            ident[:],
                        )
                    nc.vector.tensor_copy(
                        o[:, j, :],
                        pt[:, 0:NB, :].rearrange("p n w -> p (n w)"),
                    )

                # --- store per image pair
                if i % 2 == 1:
                    nc.sync.dma_start(
                        out[i - 1 : i + 1, 0].rearrange(
                            "i (j p) w -> p i j w", p=128
                        ),
                        o_pairs[i // 2][:],
                    )
    return nc


def _get_nc():
    if "nc" not in _cache:
        _cache["nc"] = _build_nc()
    return _cache["nc"]


def kernel(I, k):
    from concourse.bass_utils import run_bass_kernel_spmd

    k = int(np.asarray(k))
    assert k == K, f"kernel compiled for k={K}, got {k}"
    I = np.ascontiguousarray(np.asarray(I), dtype=np.float32)
    B = I.shape[0]
    assert I.shape == (B, C, H, W) and B == N_CORES * IMGS

    nc = _get_nc()
    in_maps = [
        {"inp": I[c * IMGS : (c + 1) * IMGS]} for c in range(N_CORES)
    ]
    res = run_bass_kernel_spmd(nc, in_maps, list(range(N_CORES))).results
    return np.concatenate([res[c]["out"] for c in range(N_CORES)], axis=0)
